# revision 6
# baseline (speedup 1.0000x reference)
"""JPEG block-DCT + quantization kernel for Trainium2 (8 NeuronCores, data parallel).

Math per image (1024x1024, 8x8 blocks):
  out[8u+v, i, j] = sum_{n,m} C[u,n] C[v,m] (img[8i+n, 8j+m] - 128) / (factor_b * Q[u,v])

Device mapping (per core: 2 images):
  - Load 128-row stripes [128, 1024] contiguously (partition = image row).
  - Stage A (PE, transpose-mode): for each 128-col chunk T [h,w]:
      At = T.T @ Dp  where Dp[8x+y, 16z+x] = C[z,y].
      At[8j'+m, 16u+i'] = sum_n C[u,n] T[8i'+n, 8j'+m]   (row-DCT + transpose, fused)
  - Stage B (ACT): PSUM->SBUF copy; u=0 columns get bias -128*sqrt(8) (the -128 centering).
  - Stage C (PE, transpose-mode): Bt = At.T @ Dp
      Bt[16u+i', 16v+j'] = full 2D-DCT coefficient for block (i', j').
  - Stage D (DVE): multiply by precomputed 1/(factor_b * Q[u,v]) tile, PSUM->SBUF,
      assembling stage_buf[(u,i'), (v, k, j')] so output rows are DRAM-contiguous.
  - Store: per u: [16 partitions, (v,j)] -> out[b, 8u+v, 16s+i', :] (512B runs).
"""

import numpy as np

import concourse.bass as bass
import concourse.mybir as mybir
import concourse.tile as tile
from concourse import bacc
from concourse.bass_utils import run_bass_kernel_spmd

NCORES = 8
B, H, W = 16, 1024, 1024
BPC = B // NCORES  # images per core

_LUM_Q = np.array([
    [16, 11, 10, 16, 24, 40, 51, 61],
    [12, 12, 14, 19, 26, 58, 60, 55],
    [14, 13, 16, 24, 40, 57, 69, 56],
    [14, 17, 22, 29, 51, 87, 80, 62],
    [18, 22, 37, 56, 68, 109, 103, 77],
    [24, 36, 55, 64, 81, 104, 113, 92],
    [49, 64, 78, 87, 103, 121, 120, 101],
    [72, 92, 95, 98, 112, 100, 103, 99]], dtype=np.float64) / 100.0

F32 = mybir.dt.float32
F32R = mybir.dt.float32r
CENTER = 128.0 * float(np.sqrt(8.0))  # DC offset after the row-DCT pass


def _dct_matrix():
    k = np.arange(8)[:, None]
    n = np.arange(8)[None, :]
    a = np.full(8, 2.0 / np.sqrt(16.0))
    a[0] = 2.0 / np.sqrt(32.0)
    return (a[:, None] * np.cos(np.pi * k * (2 * n + 1) / 16.0))  # float64 [8,8]


def _build_dp():
    """cols 0-127: Dp[8x+y, 16z+x] = C[z, y] (both DCT passes).
    cols 128-255: 128x128 identity (rhs for PE transpose mode)."""
    C = _dct_matrix().astype(np.float32)
    dp = np.zeros((128, 256), np.float32)
    x = np.arange(16)
    for z in range(8):
        for y in range(8):
            dp[8 * x + y, 16 * z + x] = C[z, y]
    dp[:, 128:256] = np.eye(128, dtype=np.float32)
    return dp


def _build_bias():
    b = np.zeros((128, 1), np.float32)
    b[:16, 0] = -CENTER
    return b


def _build_sc(factors):
    """SC[b, 16u+i', 128*kl + 16v + j'] = 1/(factor_b * Q[u,v]). [BPC, 128, 512]"""
    inv = (1.0 / (factors[:, None, None].astype(np.float64) * _LUM_Q[None]))  # [b,u,v]
    sc = np.broadcast_to(inv[:, :, None, None, :, None],
                         (len(factors), 8, 16, 4, 8, 16))
    return np.ascontiguousarray(sc.reshape(len(factors), 128, 512), dtype=np.float32)


def _emit(tc, nc, img, dp, bias, sc, out):
    # DRAM view: dims [b, u, s, i', v, j]
    out_r = out.rearrange("b (u v) (s i) j -> b u s i v j", v=8, i=16)
    with tc.tile_pool(name="const", bufs=1) as cpool, \
         tc.tile_pool(name="inp", bufs=3) as ipool, \
         tc.tile_pool(name="at", bufs=2) as apool, \
         tc.tile_pool(name="tr", bufs=2) as trpool, \
         tc.tile_pool(name="bs", bufs=2) as bspool, \
         tc.tile_pool(name="ob", bufs=3) as opool, \
         tc.tile_pool(name="pa", bufs=2, space="PSUM") as papool, \
         tc.tile_pool(name="pt1", bufs=2, space="PSUM") as pt1pool, \
         tc.tile_pool(name="pb", bufs=2, space="PSUM") as pbpool, \
         tc.tile_pool(name="pt2", bufs=2, space="PSUM") as pt2pool:
        dpt_full = cpool.tile([128, 256], F32R)
        nc.gpsimd.dma_start(dpt_full, dp)  # SWDGE cast f32 -> f32r (rounded)
        dpt = dpt_full[:, 0:128]
        ident = dpt_full[:, 128:256]
        biast = cpool.tile([128, 1], F32, tag="bias")
        nc.sync.dma_start(biast, bias)
        biasc = biast[:, 0:1]
        scts = []
        for b in range(BPC):
            sct = cpool.tile([128, 512], F32, tag=f"sc{b}")
            nc.sync.dma_start(sct, sc[b])
            scts.append(sct)

        for b in range(BPC):
            for s in range(8):
                it = ipool.tile([128, 1024], F32R)
                nc.gpsimd.dma_start(it, img[b, 0, 128 * s:128 * (s + 1), :])
                att = apool.tile([128, 1024], F32R)
                ob = opool.tile([128, 1024], F32)
                for g in range(2):
                    # MM1: A[(u,i'), w] = Dp.T @ T   (contract h; row-DCT)
                    pa = papool.tile([128, 512], F32)
                    nc.tensor.matmul(pa, dpt, it[:, 512 * g:512 * (g + 1)])
                    # B: PSUM->SBUF + center bias on u=0 partitions
                    nc.scalar.add(att[:, 512 * g:512 * (g + 1)], pa, biasc)
                for g in range(2):
                    # T1: Tr[w, (u,i')] per 128-col chunk
                    pt1 = pt1pool.tile([128, 512], F32R)
                    for kl in range(4):
                        k = 4 * g + kl
                        nc.tensor.transpose(
                            pt1[:, 128 * kl:128 * (kl + 1)],
                            att[:, 128 * k:128 * (k + 1)], ident)
                    trs = trpool.tile([128, 512], F32R)
                    nc.scalar.copy(trs, pt1)
                    # MM2: B[(v,j'), (kl,(u,i'))] = Dp.T @ Tr  (contract w; col-DCT)
                    pb = pbpool.tile([128, 512], F32)
                    nc.tensor.matmul(pb, dpt, trs)
                    bsb = bspool.tile([128, 512], F32R)
                    nc.vector.tensor_copy(bsb, pb)
                    # T2: Bt[(u,i'), (v,j')] per chunk
                    pt2 = pt2pool.tile([128, 512], F32R)
                    for kl in range(4):
                        nc.tensor.transpose(
                            pt2[:, 128 * kl:128 * (kl + 1)],
                            bsb[:, 128 * kl:128 * (kl + 1)], ident)
                    # E: scale by 1/(factor*Q), assemble (v, k, j') free layout
                    pt2_r = pt2.rearrange("p (kl v j) -> p kl v j", kl=4, v=8)
                    sc_r = scts[b].rearrange("p (kl v j) -> p kl v j", kl=4, v=8)
                    ob_r = ob.rearrange("p (v kk j) -> p kk v j", v=8, kk=8)[:, 4 * g:4 * g + 4, :, :]
                    nc.vector.tensor_mul(ob_r, pt2_r, sc_r)
                for u in range(8):
                    nc.sync.dma_start(
                        out_r[b, u, s],
                        ob[16 * u:16 * (u + 1), :].rearrange("p (v j) -> p v j", v=8))


_PROGRAM = None


def _get_program():
    global _PROGRAM
    if _PROGRAM is None:
        nc = bacc.Bacc("TRN2", target_bir_lowering=False, debug=False)
        img = nc.dram_tensor("img", [BPC, 1, H, W], F32, kind="ExternalInput").ap()
        dp = nc.dram_tensor("dp", [128, 256], F32, kind="ExternalInput").ap()
        bias = nc.dram_tensor("bias", [128, 1], F32, kind="ExternalInput").ap()
        sc = nc.dram_tensor("sc", [BPC, 128, 512], F32, kind="ExternalInput").ap()
        out = nc.dram_tensor("out", [BPC, 64, H // 8, W // 8], F32,
                             kind="ExternalOutput").ap()
        with tile.TileContext(nc) as tc:
            _emit(tc, nc, img, dp, bias, sc, out)
        nc.compile()
        _PROGRAM = nc
    return _PROGRAM


def make_in_maps(image, quality_factor):
    qf = np.asarray(quality_factor, np.float32)
    factor = np.where(qf < 50.0, 5000.0 / qf, 200.0 - 2.0 * qf).astype(np.float32)
    dpmat = _build_dp()
    image = np.ascontiguousarray(image, dtype=np.float32)
    in_maps = []
    for c in range(NCORES):
        in_maps.append({
            "img": image[BPC * c:BPC * (c + 1)],
            "dp": dpmat,
            "bias": _build_bias(),
            "sc": _build_sc(factor[BPC * c:BPC * (c + 1)]),
        })
    return in_maps


def kernel(image, quality_factor, _trace=False, _tmpdir=None):
    nc = _get_program()
    res = run_bass_kernel_spmd(nc, make_in_maps(image, quality_factor),
                               core_ids=list(range(NCORES)),
                               trace=_trace, tmpdir=_tmpdir)
    out = np.concatenate([res.results[c]["out"] for c in range(NCORES)], axis=0)
    if _trace:
        return out, res
    return out
